# revision 1
# baseline (speedup 1.0000x reference)
"""Multi-head causal attention (B=2, S=2048, D=1024, H=16, hd=64) on 8 trn2
NeuronCores.

Sharding: core c -> batch b=c//4, head-group g=c%4 (4 heads = 256 contiguous
model dims). Each core computes q/k/v projections for its head group from the
full (transposed) batch-b input, runs causal attention for its 4 heads, and
applies its slice of the output projection, producing a partial [2048, 1024]
output. The host sums the 4 partials per batch.

Matmul operands are bf16 (PE 1 cycle/row; fp32r measured ~3 cycles/row on HW);
accumulation is fp32 in PSUM. End-to-end rel err vs the fp32 reference is
~4e-3 (numpy-verified).

Scores are computed transposed (S^T[j,i] = k^T.T @ q^T) so the softmax
denominator comes free from the AV matmul via a ones-column appended to V,
and no transposes are needed anywhere. Causality is structural: only j<=i
blocks are computed; the 128x128 diagonal blocks get the (scaled) mask added.
exp() skips max-subtraction (scores are ~N(0,1); fp32 exp is safe and masked
entries underflow to exactly 0, matching the reference softmax up to
rounding).
"""

import sys

for p in ("/opt/trn_rl_repo", "/root/.axon_site/_ro/trn_rl_repo"):
    if p not in sys.path:
        sys.path.insert(0, p)

import ml_dtypes
import numpy as np

B, S, DIM, H, HD = 2, 2048, 1024, 16, 64
NCORES = 8
HG = 4  # heads per core
OG = HG * HD  # 256 output dims per core
NB = S // 512  # 4 i-blocks of 512
NJ = S // 128  # 16 j-tiles of 128

_CACHE = {}


def _build():
    import concourse.tile as tile
    from concourse import bacc, mybir

    f32 = mybir.dt.float32
    bf16 = mybir.dt.bfloat16
    Exp = mybir.ActivationFunctionType.Exp

    nc = bacc.Bacc("TRN2", target_bir_lowering=False, debug=False, num_devices=NCORES)

    xT = nc.dram_tensor("xT", [DIM, S], bf16, kind="ExternalInput")
    wqT = nc.dram_tensor("wqT", [DIM, OG], bf16, kind="ExternalInput")
    wkT = nc.dram_tensor("wkT", [DIM, OG], bf16, kind="ExternalInput")
    wvT = nc.dram_tensor("wvT", [DIM, OG], bf16, kind="ExternalInput")
    woT = nc.dram_tensor("woT", [OG, DIM], bf16, kind="ExternalInput")
    cmask8 = nc.dram_tensor("cmask8", [128, 128], f32, kind="ExternalInput")
    onesd = nc.dram_tensor("onesd", [128, 64], bf16, kind="ExternalInput")
    y = nc.dram_tensor("y", [S, DIM], f32, kind="ExternalOutput")

    xT_r = xT.ap().rearrange("(t p) s -> t p s", p=128)  # [8,128,2048]
    wqT_r = wqT.ap().rearrange("(t p) o -> t p o", p=128)  # [8,128,256]
    wkT_r = wkT.ap().rearrange("(t p) o -> t p o", p=128)
    wvT_r = wvT.ap().rearrange("(t p) o -> t p o", p=128)
    woT_r = woT.ap().rearrange("(t p) e -> t p e", p=128)  # [2,128,1024]
    y_r = y.ap().rearrange("(t p) e -> t p e", p=128)  # [16,128,1024]

    with tile.TileContext(nc) as tc:
        with (
            tc.tile_pool(name="persist", bufs=1) as pp,
            tc.tile_pool(name="work", bufs=4) as wp,
            tc.tile_pool(name="psum", bufs=4, space="PSUM") as ps,
            tc.tile_pool(name="dramp", bufs=2, space="DRAM") as dp,
        ):
            # ---- persistent SBUF tiles -------------------------------------
            xt2 = [
                [
                    pp.tile([128, 512], bf16, tag=f"xt{e}_{n}", name=f"xt{e}_{n}")
                    for n in range(4)
                ]
                for e in range(8)
            ]
            wqt = [pp.tile([128, OG], bf16, tag=f"wq{i}", name=f"wq{i}") for i in range(8)]
            wkt = [pp.tile([128, OG], bf16, tag=f"wk{i}", name=f"wk{i}") for i in range(8)]
            wvt = [pp.tile([128, OG], bf16, tag=f"wv{i}", name=f"wv{i}") for i in range(8)]
            wot = [pp.tile([128, DIM], bf16, tag=f"wo{i}", name=f"wo{i}") for i in range(2)]
            cm = pp.tile([128, 128], f32, tag="cm")
            qTp = [pp.tile([128, S], bf16, tag=f"qT{i}", name=f"qT{i}") for i in range(4)]
            kT = [pp.tile([128, S], bf16, tag=f"kT{i}", name=f"kT{i}") for i in range(2)]
            vv = [pp.tile([128, HG, HD + 1], bf16, tag=f"vv{i}", name=f"vv{i}") for i in range(NJ)]
            zT = [pp.tile([128, S], bf16, tag=f"zT{i}", name=f"zT{i}") for i in range(2)]
            ones1 = pp.tile([1, 64], bf16, tag="ones1")

            qs = [nc.sync, nc.scalar, nc.gpsimd]
            xT_c = xT.ap().rearrange("(t p) (n c) -> t p n c", p=128, c=512)
            qi = 0

            def q():
                nonlocal qi
                qi += 1
                return qs[qi % 3]

            # first q/k chains need wkt/wqt[e] + xt2[e][0], in e order
            for e in range(8):
                q().dma_start(out=wkt[e], in_=wkT_r[e])
                q().dma_start(out=wqt[e], in_=wqT_r[e])
                q().dma_start(out=xt2[e][0], in_=xT_c[e, :, 0])
            for n in range(1, 4):
                for e in range(8):
                    q().dma_start(out=xt2[e][n], in_=xT_c[e, :, n])
                    if n == 1:
                        q().dma_start(out=wvt[e], in_=wvT_r[e])
            for i in range(2):
                q().dma_start(out=wot[i], in_=woT_r[i])
            nc.gpsimd.dma_start(out=cm, in_=cmask8.ap())
            nc.gpsimd.dma_start(out=ones1, in_=onesd.ap()[0:1, :])
            for hh in range(4):
                pad = slice(64, 128) if hh % 2 == 0 else slice(0, 64)
                nc.vector.memset(qTp[hh][pad, :], 0.0)

            # ---- q/k/v projections ----------------------------------------
            # qT/kT: [o 128-chunk, s 512-block] accumulated over 8 e-tiles.
            for m in range(2):
                for n in range(NB):
                    for which, wt in (("k", wkt), ("q", wqt)):
                        acc2 = ps.tile([128, 1024], f32, tag="s2", bufs=3)
                        acc = acc2[:, 0:512]
                        for e in range(8):
                            nc.tensor.matmul(
                                acc,
                                wt[e][:, m * 128 : (m + 1) * 128],
                                xt2[e][n],
                                start=(e == 0),
                                stop=(e == 7),
                            )
                        blk = slice(n * 512, (n + 1) * 512)
                        if which == "k":
                            nc.vector.tensor_copy(kT[m][:, blk], acc)
                        else:
                            nc.scalar.copy(qTp[2 * m][0:64, blk], acc[0:64, :])
                            nc.scalar.copy(
                                qTp[2 * m + 1][64:128, blk], acc[64:128, :]
                            )
            # v: natural layout [s 128-chunk, o], with a ones column per head.
            for s in range(NJ):
                acc2 = ps.tile([128, 1024], f32, tag="s2", bufs=3)
                acc = acc2[:, 0:512]
                for e in range(8):
                    nc.tensor.matmul(
                        acc[:, 0:OG],
                        xt2[e][s // 4][:, (s % 4) * 128 : (s % 4 + 1) * 128],
                        wvt[e],
                        start=(e == 0),
                        stop=(e == 7),
                    )
                nc.vector.tensor_copy(
                    vv[s][:, :, 0:HD],
                    acc[:, 0:OG].rearrange("p (h d) -> p h d", h=HG),
                )
                nc.sync.dma_start(
                    out=vv[s][:, :, HD : HD + 1],
                    in_=onesd.ap()[:, 0:HG].rearrange("p (h o) -> p h o", o=1),
                )

            # ---- attention per head ---------------------------------------
            # The normalization of stream (h, ib) is deferred until after the
            # first block of stream (h, ib)+1 so its broadcast matmul never
            # stalls the PE waiting on the DVE denominator copy (those stalls
            # break HAM's busy window and pin the PE at 1.2 GHz).
            pending = []

            def flush_norm():
                while pending:
                    pending.pop(0)()

            for h in range(HG):
                m, po = divmod(h, 2)
                po *= 64
                for ib in range(NB):
                    psz = ps.tile([65, 512], f32, tag="z", bufs=2, name="psz")
                    nplain = 4 * ib
                    units = []

                    def mk_pair(jb, h=h, m=m, po=po, ib=ib, psz=psz):
                        def go():
                            pss = ps.tile(
                                [128, 1024], f32, tag="s2", bufs=3, name="pss"
                            )
                            for u in range(2):
                                nc.tensor.matmul(
                                    pss[:, u * 512 : (u + 1) * 512],
                                    kT[m][:, (jb + u) * 128 : (jb + u + 1) * 128],
                                    qTp[h][:, ib * 512 : (ib + 1) * 512],
                                    start=True,
                                    stop=True,
                                )
                            ex = wp.tile([128, 1024], bf16, tag="ex", name="ex", bufs=6)
                            nc.scalar.activation(ex, pss, Exp, scale=0.125)
                            for u in range(2):
                                nc.tensor.matmul(
                                    psz,
                                    vv[jb + u][:, h, :],
                                    ex[:, u * 512 : (u + 1) * 512],
                                    start=(jb + u == 0),
                                    stop=False,
                                )
                        return go

                    def mk_band(t, h=h, m=m, po=po, ib=ib, psz=psz, nplain=nplain):
                        def go():
                            jb = nplain + t
                            off = 128 * t
                            ps2 = ps.tile(
                                [128, 1024], f32, tag="s2", bufs=3, name="ps2"
                            )
                            pss = ps2[:, 0:512]
                            nc.tensor.matmul(
                                pss[:, off:512],
                                kT[m][:, jb * 128 : (jb + 1) * 128],
                                qTp[h][:, ib * 512 + off : (ib + 1) * 512],
                                start=True,
                                stop=True,
                            )
                            ex = wp.tile([128, 512], bf16, tag="exb", name="ex", bufs=6)
                            nc.vector.tensor_add(
                                pss[:, off : off + 128], pss[:, off : off + 128], cm
                            )
                            nc.scalar.activation(
                                ex[:, off:512], pss[:, off:512], Exp, scale=0.125
                            )
                            nc.tensor.matmul(
                                psz[:, off:512],
                                vv[jb][:, h, :],
                                ex[:, off:512],
                                start=(jb == 0),
                                stop=(t == 3),
                            )
                        return go

                    for jb in range(0, nplain, 2):
                        units.append(mk_pair(jb))
                    for t in range(4):
                        units.append(mk_band(t))

                    for ui, u in enumerate(units):
                        u()
                        if ui == min(2, len(units) - 1):
                            flush_norm()

                    # reciprocal of the denominator row now (DVE), broadcast
                    # across 64 partitions via SBUF->SBUF DMA; the multiply
                    # lands after the next stream's first block.
                    dn = wp.tile([1, 512], bf16, tag="dn", bufs=3, name="dn")
                    nc.vector.tensor_copy(dn, psz[64:65, :])

                    def mk_norm(m=m, po=po, ib=ib, psz=psz, dn=dn):
                        def go():
                            psb2 = ps.tile(
                                [128, 1024], f32, tag="s2", bufs=3, name="psb2"
                            )
                            nc.tensor.matmul(
                                psb2[0:64, 0:512], ones1, dn, start=True, stop=True
                            )
                            rc = wp.tile([64, 512], f32, tag="rc", bufs=3, name="rc")
                            nc.vector.reciprocal_approx_fast(rc, psb2[0:64, 0:512])
                            nc.vector.tensor_mul(
                                zT[m][po : po + 64, ib * 512 : (ib + 1) * 512],
                                psz[0:64, :],
                                rc,
                            )
                        return go

                    pending.append(mk_norm())
            flush_norm()

            # ---- output projection ----------------------------------------
            for s in range(NJ):
                ysb = wp.tile([128, DIM], f32, tag="ysb", bufs=3, name="ysb")
                for n2 in range(2):
                    psy2 = ps.tile([128, 1024], f32, tag="s2", bufs=3)
                    psy = psy2[:, 0:512]
                    for kk in range(2):
                        nc.tensor.matmul(
                            psy,
                            zT[kk][:, s * 128 : (s + 1) * 128],
                            wot[kk][:, n2 * 512 : (n2 + 1) * 512],
                            start=(kk == 0),
                            stop=(kk == 1),
                        )
                    if n2 == 0:
                        nc.scalar.copy(ysb[:, 0:512], psy)
                    else:
                        nc.vector.tensor_copy(ysb[:, 512:1024], psy)
                nc.sync.dma_start(out=y_r[s], in_=ysb)

    nc.compile()
    return nc


def _get_nc():
    if "nc" not in _CACHE:
        _CACHE["nc"] = _build()
    return _CACHE["nc"]


def _in_maps(x, mask, wq, wk, wv, wo):
    bf = ml_dtypes.bfloat16
    cm8 = np.ascontiguousarray(8.0 * np.asarray(mask)[0, 0, :128, :128].T, np.float32)
    maps = []
    for c in range(NCORES):
        b, g = divmod(c, HG)
        sl = slice(OG * g, OG * (g + 1))
        maps.append(
            {
                "xT": np.ascontiguousarray(np.asarray(x)[b].T).astype(bf),
                "wqT": np.ascontiguousarray(np.asarray(wq)[sl, :].T).astype(bf),
                "wkT": np.ascontiguousarray(np.asarray(wk)[sl, :].T).astype(bf),
                "wvT": np.ascontiguousarray(np.asarray(wv)[sl, :].T).astype(bf),
                "woT": np.ascontiguousarray(np.asarray(wo)[:, sl].T).astype(bf),
                "cmask8": cm8,
                "onesd": np.ones((128, 64), bf),
            }
        )
    return maps


def _combine(results):
    y = np.zeros((B, S, DIM), np.float32)
    for c in range(NCORES):
        y[c // HG] += results[c]["y"]
    return y


def kernel(x, mask, wq, wk, wv, wo, **run_kwargs):
    from concourse.bass_utils import run_bass_kernel_spmd

    nc = _get_nc()
    res = run_bass_kernel_spmd(
        nc, _in_maps(x, mask, wq, wk, wv, wo), core_ids=list(range(NCORES)),
        **run_kwargs,
    )
    out = _combine(res.results)
    if run_kwargs:
        _CACHE["last_result"] = res
    return out



# revision 5
# speedup vs baseline: 1.0185x; 1.0185x over previous
"""Multi-head causal attention (B=2, S=2048, D=1024, H=16, hd=64) on 8 trn2
NeuronCores.

Sharding: core c -> batch b=c//4, head-group g=c%4 (4 heads = 256 contiguous
model dims). Each core computes q/k/v projections for its head group from the
full (transposed) batch-b input, runs causal attention for its 4 heads, and
applies its slice of the output projection, producing a partial [2048, 1024]
output (bf16). The host sums the 4 partials per batch in f32.

Pipelined structure (v2): the kernel is organized as 4 sequential 512-row
i-blocks. Block n does: project k/q/v for rows 512n..512n+511 -> causal
attention for all 4 heads at i-block n -> output projection + y DMA for
i-block n-1. This starts the PE ~8us into the kernel (vs ~33us for
phase-by-phase), keeps it continuously busy (HAM clock boost), and spreads
the y writeback so there is no DMA tail.

Matmul operands are bf16 (PE 1 cycle/row); accumulation is fp32 in PSUM.
Scores are computed transposed (S^T[j,i] = k^T.T @ q^T, contracting the
head's 64 dims via partition-offset slices of kT/qT) so the softmax
denominator comes free from the AV matmul via a ones-column appended to V,
and no transposes are needed anywhere. Causality is structural: only j<=i
blocks are computed; the 128x128 diagonal blocks get the (scaled) mask added.
exp() skips max-subtraction (scores are ~N(0,1); fp32 exp is safe and masked
entries underflow to exactly 0).

Engine budget per core (@2.4GHz): PE ~100us of matmul rows, Act ~85us of exp
(the only engine with exp), DVE ~60us of copies/masks/normalization. All
copies and DMA issues are kept OFF the Act engine; band exps are merged
(2 instructions per stream instead of 4) to cut Act instruction overheads.
"""

import sys

for p in ("/opt/trn_rl_repo", "/root/.axon_site/_ro/trn_rl_repo"):
    if p not in sys.path:
        sys.path.insert(0, p)

import ml_dtypes
import numpy as np

B, S, DIM, H, HD = 2, 2048, 1024, 16, 64
NCORES = 8
HG = 4  # heads per core
OG = HG * HD  # 256 output dims per core
NB = S // 512  # 4 i-blocks of 512
NJ = S // 128  # 16 j-tiles of 128

_CACHE = {}


def _build():
    import concourse.tile as tile
    from concourse import bacc, mybir

    f32 = mybir.dt.float32
    bf16 = mybir.dt.bfloat16
    Exp = mybir.ActivationFunctionType.Exp

    nc = bacc.Bacc("TRN2", target_bir_lowering=False, debug=False, num_devices=NCORES)

    # inputs, pre-packed on host so each lands in one (or two) big DMAs
    xc = nc.dram_tensor("xc", [128, NB, 8, 512], bf16, kind="ExternalInput")
    wkq = nc.dram_tensor("wkq", [128, 8, 512], bf16, kind="ExternalInput")
    wv2 = nc.dram_tensor("wv2", [128, 8, 256], bf16, kind="ExternalInput")
    woT = nc.dram_tensor("woT", [OG, DIM], bf16, kind="ExternalInput")
    cmask8 = nc.dram_tensor("cmask8", [128, 128], f32, kind="ExternalInput")
    y = nc.dram_tensor("y", [S, DIM], bf16, kind="ExternalOutput")

    woT_r = woT.ap().rearrange("(t p) e -> t p e", p=128)  # [2,128,1024]
    y_r = y.ap().rearrange("(t p) e -> t p e", p=128)  # [16,128,1024]

    with tile.TileContext(nc) as tc:
        with (
            tc.tile_pool(name="persist", bufs=1) as pp,
            tc.tile_pool(name="work", bufs=4) as wp,
            tc.tile_pool(name="psum", bufs=4, space="PSUM") as ps,
        ):
            # ---- persistent SBUF tiles -------------------------------------
            xsb = [
                pp.tile([128, 8, 512], bf16, tag=f"x{n}", name=f"x{n}")
                for n in range(NB)
            ]
            wkqt = pp.tile([128, 8, 512], bf16, tag="wkq")
            wvt = pp.tile([128, 8, 256], bf16, tag="wv")
            wot = [pp.tile([128, DIM], bf16, tag=f"wo{i}", name=f"wo{i}") for i in range(2)]
            cm2 = pp.tile([128, 2, 128], f32, tag="cm2")
            kT = [pp.tile([128, S], bf16, tag=f"kT{i}", name=f"kT{i}") for i in range(2)]
            qT = [pp.tile([128, S], bf16, tag=f"qT{i}", name=f"qT{i}") for i in range(2)]
            vv = [pp.tile([128, HG, HD + 1], bf16, tag=f"vv{i}", name=f"vv{i}") for i in range(NJ)]
            zT = [pp.tile([128, S], bf16, tag=f"zT{i}", name=f"zT{i}") for i in range(2)]
            ones1 = pp.tile([1, 64], bf16, tag="ones1")

            sq, gq = nc.sync, nc.gpsimd

            # ---- input DMAs (2 queues, neither is PE/Act/DVE) --------------
            # xsb[2]/xsb[3] are issued later from the DVE queue (inside
            # proj_block 1/2) so their transfers don't steal HBM bandwidth
            # from the first blocks' inputs.
            sq.dma_start(out=wkqt, in_=wkq.ap())
            gq.dma_start(out=wvt, in_=wv2.ap())
            sq.dma_start(out=xsb[0], in_=xc.ap()[:, 0])
            gq.dma_start(out=cm2[:, 0, :], in_=cmask8.ap())
            gq.dma_start(out=cm2[:, 1, :], in_=cmask8.ap())
            sq.dma_start(out=xsb[1], in_=xc.ap()[:, 1])
            gq.dma_start(out=wot[0], in_=woT_r[0])
            gq.dma_start(out=wot[1], in_=woT_r[1])
            nc.vector.memset(ones1, 1.0)

            # ---- deferred softmax normalization ----------------------------
            # The normalization of stream (n, h) is deferred until after the
            # first units of stream (n, h)+1 so its broadcast matmul never
            # stalls the PE waiting on the DVE denominator copy (those stalls
            # break HAM's busy window and drop the PE clock).
            pending = []

            def flush_norm():
                while pending:
                    pending.pop(0)()

            def mk_norm(m, po, n, psz, dn):
                def go():
                    psb2 = ps.tile([128, 1024], f32, tag="s2", bufs=3, name="psb2")
                    nc.tensor.matmul(
                        psb2[0:64, 0:512], ones1, dn, start=True, stop=True
                    )
                    rc = wp.tile([64, 512], f32, tag="rc", bufs=3, name="rc")
                    nc.vector.reciprocal_approx_fast(rc, psb2[0:64, 0:512])
                    nc.vector.tensor_mul(
                        zT[m][po : po + 64, n * 512 : (n + 1) * 512],
                        psz[0:64, :],
                        rc,
                    )
                return go

            # ---- per-block projections -------------------------------------
            def proj_block(n):
                if n + 1 < NB and n >= 1:
                    # issue from the Act queue: lands after block n-1's exps,
                    # so the transfer starts once early blocks' data has
                    # mostly arrived
                    nc.scalar.dma_start(out=xsb[n + 1], in_=xc.ap()[:, n + 1])
                xn = xsb[n]
                for off_w, dst in ((0, kT), (256, qT)):
                    for m in range(2):
                        acc2 = ps.tile([128, 1024], f32, tag="s2", bufs=3, name="acc2")
                        acc = acc2[:, 0:512]
                        for e in range(8):
                            nc.tensor.matmul(
                                acc,
                                wkqt[:, e, off_w + m * 128 : off_w + (m + 1) * 128],
                                xn[:, e, :],
                                start=(e == 0),
                                stop=(e == 7),
                            )
                        nc.vector.tensor_copy(dst[m][:, n * 512 : (n + 1) * 512], acc)
                for si in range(4):
                    s = 4 * n + si
                    acc2 = ps.tile([128, 1024], f32, tag="s2", bufs=3, name="acc2v")
                    acc = acc2[:, 0:256]
                    for e in range(8):
                        nc.tensor.matmul(
                            acc,
                            xn[:, e, si * 128 : (si + 1) * 128],
                            wvt[:, e, :],
                            start=(e == 0),
                            stop=(e == 7),
                        )
                    nc.vector.tensor_copy(
                        vv[s][:, :, 0:HD],
                        acc.rearrange("p (h d) -> p h d", h=HG),
                    )
                    nc.vector.memset(vv[s][:, :, HD : HD + 1], 1.0)

            # ---- attention stream (head h, i-block n) ----------------------
            def stream(h, n):
                m, po = divmod(h, 2)
                po *= 64
                kTh = kT[m][po : po + 64, :]
                qTh = qT[m][po : po + 64, :]
                ib0 = n * 512
                psz = ps.tile([65, 512], f32, tag="z", bufs=2, name="psz")
                nplain = 4 * n
                units = []

                def mk_pair(jb):
                    def go():
                        pss = ps.tile([128, 1024], f32, tag="s2", bufs=3, name="pss")
                        for u in range(2):
                            nc.tensor.matmul(
                                pss[:, u * 512 : (u + 1) * 512],
                                kTh[:, (jb + u) * 128 : (jb + u + 1) * 128],
                                qTh[:, ib0 : ib0 + 512],
                                start=True,
                                stop=True,
                            )
                        ex = wp.tile([128, 1024], bf16, tag="ex", name="ex", bufs=6)
                        nc.scalar.activation(ex, pss, Exp, scale=0.125)
                        for u in range(2):
                            nc.tensor.matmul(
                                psz,
                                vv[jb + u][:, h, :],
                                ex[:, u * 512 : (u + 1) * 512],
                                start=(jb + u == 0),
                                stop=False,
                            )
                    return go

                def mk_band_a():
                    # diagonal j-tiles t=0,1 merged into one psum tile/exp
                    def go():
                        jb = nplain
                        A = ps.tile([128, 1024], f32, tag="s2", bufs=3, name="A")
                        nc.tensor.matmul(
                            A[:, 0:512],
                            kTh[:, jb * 128 : (jb + 1) * 128],
                            qTh[:, ib0 : ib0 + 512],
                            start=True,
                            stop=True,
                        )
                        nc.tensor.matmul(
                            A[:, 512:896],
                            kTh[:, (jb + 1) * 128 : (jb + 2) * 128],
                            qTh[:, ib0 + 128 : ib0 + 512],
                            start=True,
                            stop=True,
                        )
                        Av = A.rearrange("p (t c) -> p t c", c=512)[:, :, 0:128]
                        nc.vector.tensor_add(Av, Av, cm2)
                        exA = wp.tile([128, 896], bf16, tag="exb", name="exA", bufs=4)
                        nc.scalar.activation(exA, A[:, 0:896], Exp, scale=0.125)
                        nc.tensor.matmul(
                            psz,
                            vv[jb][:, h, :],
                            exA[:, 0:512],
                            start=(jb == 0),
                            stop=False,
                        )
                        nc.tensor.matmul(
                            psz[:, 128:512],
                            vv[jb + 1][:, h, :],
                            exA[:, 512:896],
                            start=False,
                            stop=False,
                        )
                    return go

                def mk_band_b():
                    # diagonal j-tiles t=2,3 merged into one psum tile/exp
                    def go():
                        jb = nplain + 2
                        Bt = ps.tile([128, 1024], f32, tag="s2", bufs=3, name="Bt")
                        nc.tensor.matmul(
                            Bt[:, 0:256],
                            kTh[:, jb * 128 : (jb + 1) * 128],
                            qTh[:, ib0 + 256 : ib0 + 512],
                            start=True,
                            stop=True,
                        )
                        nc.tensor.matmul(
                            Bt[:, 256:384],
                            kTh[:, (jb + 1) * 128 : (jb + 2) * 128],
                            qTh[:, ib0 + 384 : ib0 + 512],
                            start=True,
                            stop=True,
                        )
                        Bv = Bt[:, 0:512].rearrange("p (t c) -> p t c", c=256)[
                            :, :, 0:128
                        ]
                        nc.vector.tensor_add(Bv, Bv, cm2)
                        exB = wp.tile([128, 896], bf16, tag="exb", name="exB", bufs=4)
                        nc.scalar.activation(exB[:, 0:384], Bt[:, 0:384], Exp, scale=0.125)
                        nc.tensor.matmul(
                            psz[:, 256:512],
                            vv[jb][:, h, :],
                            exB[:, 0:256],
                            start=False,
                            stop=False,
                        )
                        nc.tensor.matmul(
                            psz[:, 384:512],
                            vv[jb + 1][:, h, :],
                            exB[:, 256:384],
                            start=False,
                            stop=True,
                        )
                    return go

                for jb in range(0, nplain, 2):
                    units.append(mk_pair(jb))
                units.append(mk_band_a())
                units.append(mk_band_b())

                for ui, u in enumerate(units):
                    u()
                    if ui == min(2, len(units) - 1):
                        flush_norm()

                # denominator row out of PSUM now (DVE); reciprocal+multiply
                # land after the next stream's first units (see mk_norm).
                dn = wp.tile([1, 512], bf16, tag="dn", bufs=3, name="dn")
                nc.vector.tensor_copy(dn, psz[64:65, :])
                pending.append(mk_norm(m, po, n, psz, dn))

            # ---- output projection for one 128-row s-tile ------------------
            def oproj(s):
                psy2 = ps.tile([128, 1024], f32, tag="s2", bufs=3, name="psy2")
                for n2 in range(2):
                    for kk in range(2):
                        nc.tensor.matmul(
                            psy2[:, n2 * 512 : (n2 + 1) * 512],
                            zT[kk][:, s * 128 : (s + 1) * 128],
                            wot[kk][:, n2 * 512 : (n2 + 1) * 512],
                            start=(kk == 0),
                            stop=(kk == 1),
                        )
                ysb = wp.tile([128, DIM], bf16, tag="ysb", bufs=3, name="ysb")
                nc.scalar.copy(ysb[:, 0:512], psy2[:, 0:512])
                nc.vector.tensor_copy(ysb[:, 512:1024], psy2[:, 512:1024])
                (sq if s % 2 == 0 else gq).dma_start(out=y_r[s], in_=ysb)

            # ---- main pipelined loop ---------------------------------------
            for n in range(NB):
                proj_block(n)
                for h in range(HG):
                    stream(h, n)
                    if n > 0:
                        oproj(4 * (n - 1) + h)
            flush_norm()
            for k in range(4):
                oproj(12 + k)

    nc.compile()
    return nc


def _get_nc():
    if "nc" not in _CACHE:
        _CACHE["nc"] = _build()
    return _CACHE["nc"]


def _in_maps(x, mask, wq, wk, wv, wo):
    bf = ml_dtypes.bfloat16
    cm8 = np.ascontiguousarray(8.0 * np.asarray(mask)[0, 0, :128, :128].T, np.float32)
    maps = []
    for c in range(NCORES):
        b, g = divmod(c, HG)
        sl = slice(OG * g, OG * (g + 1))
        # xc[p, n, e, c] = x[b][512n+c, 128e+p]
        xT = np.asarray(x)[b].T.astype(bf)  # [1024, 2048]
        xc = np.ascontiguousarray(
            xT.reshape(8, 128, NB, 512).transpose(1, 2, 0, 3)
        )
        # wkq[p, e, 0:256] = wk[sl].T chunk e; [256:512] = wq[sl].T chunk e
        wkT = np.asarray(wk)[sl, :].T.astype(bf)  # [1024, 256]
        wqT = np.asarray(wq)[sl, :].T.astype(bf)
        wkq = np.ascontiguousarray(
            np.concatenate([wkT, wqT], axis=1).reshape(8, 128, 512).transpose(1, 0, 2)
        )
        wvT = np.asarray(wv)[sl, :].T.astype(bf)  # [1024, 256]
        wv2 = np.ascontiguousarray(wvT.reshape(8, 128, 256).transpose(1, 0, 2))
        maps.append(
            {
                "xc": xc,
                "wkq": wkq,
                "wv2": wv2,
                "woT": np.ascontiguousarray(np.asarray(wo)[:, sl].T).astype(bf),
                "cmask8": cm8,
            }
        )
    return maps


def _combine(results):
    y = np.zeros((B, S, DIM), np.float32)
    for c in range(NCORES):
        y[c // HG] += results[c]["y"].astype(np.float32)
    return y


def kernel(x, mask, wq, wk, wv, wo, **run_kwargs):
    from concourse.bass_utils import run_bass_kernel_spmd

    nc = _get_nc()
    res = run_bass_kernel_spmd(
        nc, _in_maps(x, mask, wq, wk, wv, wo), core_ids=list(range(NCORES)),
        **run_kwargs,
    )
    out = _combine(res.results)
    if run_kwargs:
        _CACHE["last_result"] = res
    return out


# revision 6
# speedup vs baseline: 1.0567x; 1.0375x over previous
"""Multi-head causal attention (B=2, S=2048, D=1024, H=16, hd=64) on 8 trn2
NeuronCores.

Sharding: core c -> batch b=c//4, head-group g=c%4 (4 heads = 256 contiguous
model dims). Each core computes q/k/v projections for its head group from the
full (transposed) batch-b input, runs causal attention for its 4 heads, and
applies its slice of the output projection, producing a partial [2048, 1024]
output (bf16). The host sums the 4 partials per batch in f32.

Pipelined structure (v3): work is organized around 4 sequential 512-row
i-blocks. Block 0's projections run standalone at the start (fed by
fine-grained e-chunk DMAs so the PE starts ~2us in); thereafter the
projection units for block n+1 are INTERLEAVED into the attention streams of
block n, and the output projection + y DMA for block n-1 runs between
streams. This keeps the PE continuously busy (attention alone is
Act/exp-bound; projection units fill the bubbles), which also holds the HAM
clock boost at 2.4GHz, and spreads input/output DMA across the whole kernel.

Matmul operands are bf16 (PE 1 cycle/row); accumulation is fp32 in PSUM.
Scores are computed transposed (S^T[j,i] = k^T.T @ q^T, contracting the
head's 64 dims via partition-offset slices of kT/qT) so the softmax
denominator comes free from the AV matmul via a ones-column appended to V,
and no transposes are needed anywhere. Causality is structural: only j<=i
blocks are computed; the 128x128 diagonal blocks get the (scaled) mask added.
exp() skips max-subtraction (scores are ~N(0,1); fp32 exp is safe and masked
entries underflow to exactly 0). The two diagonal-band score tiles share one
PSUM tile and one exp instruction per pair (Act instruction overhead is the
attention-phase limiter). All copies and DMA issues are kept OFF the Act
engine except the staggered xsb[2]/xsb[3] loads, whose position in the Act
queue delays their transfer until the early blocks' inputs have landed.
"""

import sys

for p in ("/opt/trn_rl_repo", "/root/.axon_site/_ro/trn_rl_repo"):
    if p not in sys.path:
        sys.path.insert(0, p)

import ml_dtypes
import numpy as np

B, S, DIM, H, HD = 2, 2048, 1024, 16, 64
NCORES = 8
HG = 4  # heads per core
OG = HG * HD  # 256 output dims per core
NB = S // 512  # 4 i-blocks of 512
NJ = S // 128  # 16 j-tiles of 128

_CACHE = {}


def _build():
    import concourse.tile as tile
    from concourse import bacc, mybir

    f32 = mybir.dt.float32
    bf16 = mybir.dt.bfloat16
    Exp = mybir.ActivationFunctionType.Exp

    nc = bacc.Bacc("TRN2", target_bir_lowering=False, debug=False, num_devices=NCORES)

    # inputs, pre-packed on host (see _in_maps for layouts)
    xc = nc.dram_tensor("xc", [128, NB, 8, 512], bf16, kind="ExternalInput")
    wkq = nc.dram_tensor("wkq", [128, 4, 8, 128], bf16, kind="ExternalInput")
    wv2 = nc.dram_tensor("wv2", [128, 8, 256], bf16, kind="ExternalInput")
    woT = nc.dram_tensor("woT", [OG, DIM], bf16, kind="ExternalInput")
    cmask8 = nc.dram_tensor("cmask8", [128, 128], f32, kind="ExternalInput")
    y = nc.dram_tensor("y", [S, DIM], bf16, kind="ExternalOutput")

    woT_r = woT.ap().rearrange("(t p) e -> t p e", p=128)  # [2,128,1024]
    y_r = y.ap().rearrange("(t p) e -> t p e", p=128)  # [16,128,1024]

    with tile.TileContext(nc) as tc:
        with (
            tc.tile_pool(name="persist", bufs=1) as pp,
            tc.tile_pool(name="work", bufs=4) as wp,
            tc.tile_pool(name="psum", bufs=4, space="PSUM") as ps,
        ):
            # ---- persistent SBUF tiles -------------------------------------
            xsb = [
                pp.tile([128, 8, 512], bf16, tag=f"x{n}", name=f"x{n}")
                for n in range(NB)
            ]
            wkqt = pp.tile([128, 4, 8, 128], bf16, tag="wkq")
            wvt = pp.tile([128, 8, 256], bf16, tag="wv")
            wot = [pp.tile([128, DIM], bf16, tag=f"wo{i}", name=f"wo{i}") for i in range(2)]
            cm2 = pp.tile([128, 2, 128], f32, tag="cm2")
            kT = [pp.tile([128, S], bf16, tag=f"kT{i}", name=f"kT{i}") for i in range(2)]
            qT = [pp.tile([128, S], bf16, tag=f"qT{i}", name=f"qT{i}") for i in range(2)]
            vv = [pp.tile([128, HG, HD + 1], bf16, tag=f"vv{i}", name=f"vv{i}") for i in range(NJ)]
            zT = [pp.tile([128, S], bf16, tag=f"zT{i}", name=f"zT{i}") for i in range(2)]
            ones1 = pp.tile([1, 64], bf16, tag="ones1")

            sq, gq = nc.sync, nc.gpsimd

            # ---- input DMAs ------------------------------------------------
            # First two x blocks arrive as 8 per-e chunks each so the first
            # projection accumulations can trail the transfers; later blocks
            # are single 1MB DMAs issued from the Act queue mid-kernel.
            sq.dma_start(out=wkqt[:, 0], in_=wkq.ap()[:, 0])  # k m=0
            for e in range(0, 8, 2):
                sq.dma_start(out=xsb[0][:, e, :], in_=xc.ap()[:, 0, e, :])
                gq.dma_start(out=xsb[0][:, e + 1, :], in_=xc.ap()[:, 0, e + 1, :])
            gq.dma_start(out=wkqt[:, 1], in_=wkq.ap()[:, 1])  # k m=1
            sq.dma_start(out=wkqt[:, 2], in_=wkq.ap()[:, 2])  # q m=0
            gq.dma_start(out=wkqt[:, 3], in_=wkq.ap()[:, 3])  # q m=1
            sq.dma_start(out=wvt, in_=wv2.ap())
            gq.dma_start(out=cm2[:, 0, :], in_=cmask8.ap())
            sq.dma_start(out=cm2[:, 1, :], in_=cmask8.ap())
            for e in range(0, 8, 2):
                gq.dma_start(out=xsb[1][:, e, :], in_=xc.ap()[:, 1, e, :])
                sq.dma_start(out=xsb[1][:, e + 1, :], in_=xc.ap()[:, 1, e + 1, :])
            gq.dma_start(out=wot[0], in_=woT_r[0])
            sq.dma_start(out=wot[1], in_=woT_r[1])
            nc.vector.memset(ones1, 1.0)

            # ---- deferred softmax normalization ----------------------------
            # The normalization of stream (n, h) is deferred until after the
            # first units of the next stream so its broadcast matmul never
            # stalls the PE waiting on the DVE denominator copy.
            pending = []

            def flush_norm():
                while pending:
                    pending.pop(0)()

            def mk_norm(m, po, n, psz, dn):
                def go():
                    psb2 = ps.tile([128, 1024], f32, tag="s2", bufs=3, name="psb2")
                    nc.tensor.matmul(
                        psb2[0:64, 0:512], ones1, dn, start=True, stop=True
                    )
                    rc = wp.tile([64, 512], f32, tag="rc", bufs=3, name="rc")
                    nc.vector.reciprocal_approx_fast(rc, psb2[0:64, 0:512])
                    nc.vector.tensor_mul(
                        zT[m][po : po + 64, n * 512 : (n + 1) * 512],
                        psz[0:64, :],
                        rc,
                    )
                return go

            # ---- projection units for one block (8 closures) ---------------
            def proj_units(n):
                xn = xsb[n]
                units = []

                def mk_kq(g):
                    # g: 0 = k m0, 1 = k m1, 2 = q m0, 3 = q m1
                    def go():
                        dst = kT[g % 2] if g < 2 else qT[g % 2]
                        acc2 = ps.tile([128, 1024], f32, tag="s2", bufs=3, name="acc2")
                        acc = acc2[:, 0:512]
                        for e in range(8):
                            nc.tensor.matmul(
                                acc,
                                wkqt[:, g, e, :],
                                xn[:, e, :],
                                start=(e == 0),
                                stop=(e == 7),
                            )
                        nc.vector.tensor_copy(dst[:, n * 512 : (n + 1) * 512], acc)
                    return go

                def mk_v(si):
                    def go():
                        s = 4 * n + si
                        acc2 = ps.tile([128, 1024], f32, tag="s2", bufs=3, name="acc2v")
                        acc = acc2[:, 0:256]
                        for e in range(8):
                            nc.tensor.matmul(
                                acc,
                                xn[:, e, si * 128 : (si + 1) * 128],
                                wvt[:, e, :],
                                start=(e == 0),
                                stop=(e == 7),
                            )
                        nc.vector.tensor_copy(
                            vv[s][:, :, 0:HD],
                            acc.rearrange("p (h d) -> p h d", h=HG),
                        )
                        nc.vector.memset(vv[s][:, :, HD : HD + 1], 1.0)
                    return go

                units.append(mk_kq(0))
                units.append(mk_kq(1))
                units.append(mk_kq(2))
                units.append(mk_kq(3))
                for si in range(4):
                    units.append(mk_v(si))
                return units

            # ---- attention stream (head h, i-block n) ----------------------
            def stream(h, n, inject):
                m, po = divmod(h, 2)
                po *= 64
                kTh = kT[m][po : po + 64, :]
                qTh = qT[m][po : po + 64, :]
                ib0 = n * 512
                psz = ps.tile([65, 512], f32, tag="z", bufs=2, name="psz")
                nplain = 4 * n
                units = []

                def mk_pair(jb):
                    def go():
                        pss = ps.tile([128, 1024], f32, tag="s2", bufs=3, name="pss")
                        for u in range(2):
                            nc.tensor.matmul(
                                pss[:, u * 512 : (u + 1) * 512],
                                kTh[:, (jb + u) * 128 : (jb + u + 1) * 128],
                                qTh[:, ib0 : ib0 + 512],
                                start=True,
                                stop=True,
                            )
                        ex = wp.tile([128, 1024], bf16, tag="ex", name="ex", bufs=6)
                        nc.scalar.activation(ex, pss, Exp, scale=0.125)
                        for u in range(2):
                            nc.tensor.matmul(
                                psz,
                                vv[jb + u][:, h, :],
                                ex[:, u * 512 : (u + 1) * 512],
                                start=(jb + u == 0),
                                stop=False,
                            )
                    return go

                def mk_band_a():
                    # diagonal j-tiles t=0,1 merged into one psum tile/exp
                    def go():
                        jb = nplain
                        A = ps.tile([128, 1024], f32, tag="s2", bufs=3, name="A")
                        nc.tensor.matmul(
                            A[:, 0:512],
                            kTh[:, jb * 128 : (jb + 1) * 128],
                            qTh[:, ib0 : ib0 + 512],
                            start=True,
                            stop=True,
                        )
                        nc.tensor.matmul(
                            A[:, 512:896],
                            kTh[:, (jb + 1) * 128 : (jb + 2) * 128],
                            qTh[:, ib0 + 128 : ib0 + 512],
                            start=True,
                            stop=True,
                        )
                        Av = A.rearrange("p (t c) -> p t c", c=512)[:, :, 0:128]
                        nc.vector.tensor_add(Av, Av, cm2)
                        exA = wp.tile([128, 896], bf16, tag="exb", name="exA", bufs=4)
                        nc.scalar.activation(exA, A[:, 0:896], Exp, scale=0.125)
                        nc.tensor.matmul(
                            psz,
                            vv[jb][:, h, :],
                            exA[:, 0:512],
                            start=(jb == 0),
                            stop=False,
                        )
                        nc.tensor.matmul(
                            psz[:, 128:512],
                            vv[jb + 1][:, h, :],
                            exA[:, 512:896],
                            start=False,
                            stop=False,
                        )
                    return go

                def mk_band_b():
                    # diagonal j-tiles t=2,3 merged into one psum tile/exp
                    def go():
                        jb = nplain + 2
                        Bt = ps.tile([128, 1024], f32, tag="s2", bufs=3, name="Bt")
                        nc.tensor.matmul(
                            Bt[:, 0:256],
                            kTh[:, jb * 128 : (jb + 1) * 128],
                            qTh[:, ib0 + 256 : ib0 + 512],
                            start=True,
                            stop=True,
                        )
                        nc.tensor.matmul(
                            Bt[:, 256:384],
                            kTh[:, (jb + 1) * 128 : (jb + 2) * 128],
                            qTh[:, ib0 + 384 : ib0 + 512],
                            start=True,
                            stop=True,
                        )
                        Bv = Bt[:, 0:512].rearrange("p (t c) -> p t c", c=256)[
                            :, :, 0:128
                        ]
                        nc.vector.tensor_add(Bv, Bv, cm2)
                        exB = wp.tile([128, 896], bf16, tag="exb", name="exB", bufs=4)
                        nc.scalar.activation(exB[:, 0:384], Bt[:, 0:384], Exp, scale=0.125)
                        nc.tensor.matmul(
                            psz[:, 256:512],
                            vv[jb][:, h, :],
                            exB[:, 0:256],
                            start=False,
                            stop=False,
                        )
                        nc.tensor.matmul(
                            psz[:, 384:512],
                            vv[jb + 1][:, h, :],
                            exB[:, 256:384],
                            start=False,
                            stop=True,
                        )
                    return go

                for jb in range(0, nplain, 2):
                    units.append(mk_pair(jb))
                units.append(mk_band_a())
                units.append(mk_band_b())

                for ui, u in enumerate(units):
                    u()
                    inject()
                    if ui == min(2, len(units) - 1):
                        flush_norm()

                # denominator row out of PSUM now (DVE); reciprocal+multiply
                # land after the next stream's first units (see mk_norm).
                dn = wp.tile([1, 512], bf16, tag="dn", bufs=3, name="dn")
                nc.vector.tensor_copy(dn, psz[64:65, :])
                pending.append(mk_norm(m, po, n, psz, dn))

            # ---- output projection for one 128-row s-tile ------------------
            def oproj(s):
                psy2 = ps.tile([128, 1024], f32, tag="s2", bufs=3, name="psy2")
                for n2 in range(2):
                    for kk in range(2):
                        nc.tensor.matmul(
                            psy2[:, n2 * 512 : (n2 + 1) * 512],
                            zT[kk][:, s * 128 : (s + 1) * 128],
                            wot[kk][:, n2 * 512 : (n2 + 1) * 512],
                            start=(kk == 0),
                            stop=(kk == 1),
                        )
                ysb = wp.tile([128, DIM], bf16, tag="ysb", bufs=3, name="ysb")
                nc.scalar.copy(ysb[:, 0:512], psy2[:, 0:512])
                nc.vector.tensor_copy(ysb[:, 512:1024], psy2[:, 512:1024])
                (sq if s % 2 == 0 else gq).dma_start(out=y_r[s], in_=ysb)

            # ---- main pipelined loop ---------------------------------------
            for u in proj_units(0):
                u()

            for n in range(NB):
                # proj units of block n+1 get injected into this block's
                # attention streams, spread evenly across the units
                nxt = list(proj_units(n + 1)) if n + 1 < NB else []
                if n + 2 < NB:
                    xn2 = n + 2

                    def mk_dma(xn2=xn2):
                        def go():
                            nc.scalar.dma_start(
                                out=xsb[xn2], in_=xc.ap()[:, xn2]
                            )
                        return go

                    nxt.insert(0, mk_dma())
                n_units = 4 * (2 * n + 2)
                frac = [0.0]

                def inject(nxt=nxt, n_units=n_units, frac=frac, total=len(nxt)):
                    frac[0] += total / n_units
                    while nxt and frac[0] >= 1.0:
                        frac[0] -= 1.0
                        nxt.pop(0)()

                for h in range(HG):
                    stream(h, n, inject)
                    if n > 0:
                        oproj(4 * (n - 1) + h)
                while nxt:
                    nxt.pop(0)()
            flush_norm()
            for k in range(4):
                oproj(12 + k)

    nc.compile()
    return nc


def _get_nc():
    if "nc" not in _CACHE:
        _CACHE["nc"] = _build()
    return _CACHE["nc"]


def _in_maps(x, mask, wq, wk, wv, wo):
    bf = ml_dtypes.bfloat16
    cm8 = np.ascontiguousarray(8.0 * np.asarray(mask)[0, 0, :128, :128].T, np.float32)
    maps = []
    for c in range(NCORES):
        b, g = divmod(c, HG)
        sl = slice(OG * g, OG * (g + 1))
        # xc[p, n, e, c] = x[b][512n+c, 128e+p]
        xT = np.asarray(x)[b].T.astype(bf)  # [1024, 2048]
        xcm = np.ascontiguousarray(
            xT.reshape(8, 128, NB, 512).transpose(1, 2, 0, 3)
        )
        # wkq[p, g, e, :]: g0/g1 = wk m-chunks, g2/g3 = wq m-chunks
        wkT = np.asarray(wk)[sl, :].T.astype(bf)  # [1024, 256]
        wqT = np.asarray(wq)[sl, :].T.astype(bf)
        wkqm = np.ascontiguousarray(
            np.stack(
                [wkT[:, 0:128], wkT[:, 128:256], wqT[:, 0:128], wqT[:, 128:256]],
                axis=0,
            )
            .reshape(4, 8, 128, 128)
            .transpose(2, 0, 1, 3)
        )
        wvT = np.asarray(wv)[sl, :].T.astype(bf)  # [1024, 256]
        wv2m = np.ascontiguousarray(wvT.reshape(8, 128, 256).transpose(1, 0, 2))
        maps.append(
            {
                "xc": xcm,
                "wkq": wkqm,
                "wv2": wv2m,
                "woT": np.ascontiguousarray(np.asarray(wo)[:, sl].T).astype(bf),
                "cmask8": cm8,
            }
        )
    return maps


def _combine(results):
    y = np.zeros((B, S, DIM), np.float32)
    for c in range(NCORES):
        y[c // HG] += results[c]["y"].astype(np.float32)
    return y


def kernel(x, mask, wq, wk, wv, wo, **run_kwargs):
    from concourse.bass_utils import run_bass_kernel_spmd

    nc = _get_nc()
    res = run_bass_kernel_spmd(
        nc, _in_maps(x, mask, wq, wk, wv, wo), core_ids=list(range(NCORES)),
        **run_kwargs,
    )
    out = _combine(res.results)
    if run_kwargs:
        _CACHE["last_result"] = res
    return out


# revision 7
# speedup vs baseline: 1.1045x; 1.0453x over previous
"""Multi-head causal attention (B=2, S=2048, D=1024, H=16, hd=64) on 8 trn2
NeuronCores.

Sharding: core c -> batch b=c//4, head-group g=c%4 (4 heads = 256 contiguous
model dims). Each core computes q/k/v projections for its head group from the
full (transposed) batch-b input, runs causal attention for its 4 heads, and
applies its slice of the output projection, producing a partial [2048, 1024]
output (bf16). The host sums the 4 partials per batch in f32.

Pipelined structure (v4): work is organized around 4 sequential 512-row
i-blocks. Block 0's projections run standalone at the start; thereafter the
projection units for block n+1 are INTERLEAVED into the attention streams of
block n, and output-projection tiles run one stream late (so their zT inputs
never stall the PE). Within and across attention streams the AV matmuls lag
the QK matmuls by one unit, hiding each unit's exp() latency behind the next
unit's score matmuls. This keeps the PE continuously busy, which also holds
the HAM clock boost at 2.4GHz.

DMA: the first two x blocks + all qkv weights are issued upfront on the
SP/Pool queues (the ~3.6MB the first two blocks need); x blocks 2/3 and the
wo tiles are issued from the Act queue mid-stream so their transfers can't
steal HBM bandwidth from earlier-needed data. y tiles stream out per block.

Matmul operands are bf16 (PE 1 cycle/row); accumulation is fp32 in PSUM.
Scores are computed transposed (S^T[j,i] = k^T.T @ q^T, contracting the
head's 64 dims via partition-offset slices of kT/qT) so the softmax
denominator comes free from the AV matmul via a ones-column appended to V,
and no transposes are needed anywhere. Causality is structural: only j<=i
blocks are computed; the 128x128 diagonal blocks get the (scaled) mask added.
exp() skips max-subtraction (scores are ~N(0,1); fp32 exp is safe and masked
entries underflow to exactly 0). The two diagonal-band score tiles share one
PSUM tile and one exp instruction per pair (Act instruction overhead is the
attention-phase limiter). All copies and DMA issues are kept OFF the Act
engine except the staggered mid-kernel loads.
"""

import sys

for p in ("/opt/trn_rl_repo", "/root/.axon_site/_ro/trn_rl_repo"):
    if p not in sys.path:
        sys.path.insert(0, p)

import ml_dtypes
import numpy as np

B, S, DIM, H, HD = 2, 2048, 1024, 16, 64
NCORES = 8
HG = 4  # heads per core
OG = HG * HD  # 256 output dims per core
NB = S // 512  # 4 i-blocks of 512
NJ = S // 128  # 16 j-tiles of 128

_CACHE = {}


def _build():
    import concourse.tile as tile
    from concourse import bacc, mybir

    f32 = mybir.dt.float32
    bf16 = mybir.dt.bfloat16
    Exp = mybir.ActivationFunctionType.Exp

    nc = bacc.Bacc("TRN2", target_bir_lowering=False, debug=False, num_devices=NCORES)

    # inputs, pre-packed on host (see _in_maps for layouts)
    xc = nc.dram_tensor("xc", [128, NB, 8, 512], bf16, kind="ExternalInput")
    wkq = nc.dram_tensor("wkq", [128, 4, 8, 128], bf16, kind="ExternalInput")
    wv2 = nc.dram_tensor("wv2", [128, 8, 256], bf16, kind="ExternalInput")
    woT = nc.dram_tensor("woT", [OG, DIM], bf16, kind="ExternalInput")
    cmask8 = nc.dram_tensor("cmask8", [128, 128], f32, kind="ExternalInput")
    y = nc.dram_tensor("y", [S, DIM], bf16, kind="ExternalOutput")

    woT_r = woT.ap().rearrange("(t p) e -> t p e", p=128)  # [2,128,1024]
    y_r = y.ap().rearrange("(t p) e -> t p e", p=128)  # [16,128,1024]

    with tile.TileContext(nc) as tc:
        with (
            tc.tile_pool(name="persist", bufs=1) as pp,
            tc.tile_pool(name="work", bufs=4) as wp,
            tc.tile_pool(name="psum", bufs=4, space="PSUM") as ps,
        ):
            # ---- persistent SBUF tiles -------------------------------------
            xsb = [
                pp.tile([128, 8, 512], bf16, tag=f"x{n}", name=f"x{n}")
                for n in range(NB)
            ]
            wkqt = pp.tile([128, 4, 8, 128], bf16, tag="wkq")
            wvt = pp.tile([128, 8, 256], bf16, tag="wv")
            wot = [pp.tile([128, DIM], bf16, tag=f"wo{i}", name=f"wo{i}") for i in range(2)]
            cm2 = pp.tile([128, 2, 128], f32, tag="cm2")
            kT = [pp.tile([128, S], bf16, tag=f"kT{i}", name=f"kT{i}") for i in range(2)]
            qT = [pp.tile([128, S], bf16, tag=f"qT{i}", name=f"qT{i}") for i in range(2)]
            vv = [pp.tile([128, HG, HD + 1], bf16, tag=f"vv{i}", name=f"vv{i}") for i in range(NJ)]
            zT = [pp.tile([128, S], bf16, tag=f"zT{i}", name=f"zT{i}") for i in range(2)]
            ones1 = pp.tile([1, 64], bf16, tag="ones1")

            sq, gq = nc.sync, nc.gpsimd

            # ---- upfront input DMAs (priority set: blocks 0/1 + weights) ---
            sq.dma_start(out=wkqt[:, 0], in_=wkq.ap()[:, 0])  # k m=0
            gq.dma_start(out=wkqt[:, 1], in_=wkq.ap()[:, 1])  # k m=1
            sq.dma_start(out=xsb[0][:, 0:4, :], in_=xc.ap()[:, 0, 0:4, :])
            gq.dma_start(out=xsb[0][:, 4:8, :], in_=xc.ap()[:, 0, 4:8, :])
            sq.dma_start(out=wkqt[:, 2], in_=wkq.ap()[:, 2])  # q m=0
            gq.dma_start(out=wkqt[:, 3], in_=wkq.ap()[:, 3])  # q m=1
            sq.dma_start(out=xsb[1][:, 0:4, :], in_=xc.ap()[:, 1, 0:4, :])
            gq.dma_start(out=wvt, in_=wv2.ap())
            sq.dma_start(out=cm2[:, 0, :], in_=cmask8.ap())
            gq.dma_start(out=xsb[1][:, 4:8, :], in_=xc.ap()[:, 1, 4:8, :])
            sq.dma_start(out=cm2[:, 1, :], in_=cmask8.ap())
            nc.vector.memset(ones1, 1.0)

            # ---- deferred softmax normalization ----------------------------
            pending = []

            def flush_norm():
                while pending:
                    pending.pop(0)()

            def mk_norm(m, po, n, psz, dn):
                def go():
                    psb2 = ps.tile([128, 1024], f32, tag="s2", bufs=3, name="psb2")
                    nc.tensor.matmul(
                        psb2[0:64, 0:512], ones1, dn, start=True, stop=True
                    )
                    rc = wp.tile([64, 512], f32, tag="rc", bufs=3, name="rc")
                    nc.vector.reciprocal_approx_fast(rc, psb2[0:64, 0:512])
                    nc.vector.tensor_mul(
                        zT[m][po : po + 64, n * 512 : (n + 1) * 512],
                        psz[0:64, :],
                        rc,
                    )
                return go

            # ---- projection units for one block (8 closures) ---------------
            def proj_units(n):
                xn = xsb[n]
                units = []

                def mk_kq(g):
                    # g: 0 = k m0, 1 = k m1, 2 = q m0, 3 = q m1
                    def go():
                        dst = kT[g % 2] if g < 2 else qT[g % 2]
                        acc2 = ps.tile([128, 1024], f32, tag="s2", bufs=3, name="acc2")
                        acc = acc2[:, 0:512]
                        for e in range(8):
                            nc.tensor.matmul(
                                acc,
                                wkqt[:, g, e, :],
                                xn[:, e, :],
                                start=(e == 0),
                                stop=(e == 7),
                            )
                        nc.vector.tensor_copy(dst[:, n * 512 : (n + 1) * 512], acc)
                    return go

                def mk_v(si):
                    def go():
                        s = 4 * n + si
                        acc2 = ps.tile([128, 1024], f32, tag="s2", bufs=3, name="acc2v")
                        acc = acc2[:, 0:256]
                        for e in range(8):
                            nc.tensor.matmul(
                                acc,
                                xn[:, e, si * 128 : (si + 1) * 128],
                                wvt[:, e, :],
                                start=(e == 0),
                                stop=(e == 7),
                            )
                        nc.vector.tensor_copy(
                            vv[s][:, :, 0:HD],
                            acc.rearrange("p (h d) -> p h d", h=HG),
                        )
                        nc.vector.memset(vv[s][:, :, HD : HD + 1], 1.0)
                    return go

                for g in range(4):
                    units.append(mk_kq(g))
                for si in range(4):
                    units.append(mk_v(si))
                return units

            # ---- attention stream (head h, i-block n) ----------------------
            # carry: closures handed from the previous stream (its last AV +
            # denominator copy), run after this stream's first QK so the PE
            # never waits on the previous stream's final exp.
            carry = []

            def run_carry():
                while carry:
                    carry.pop(0)()

            def stream(h, n, inject):
                m, po = divmod(h, 2)
                po *= 64
                kTh = kT[m][po : po + 64, :]
                qTh = qT[m][po : po + 64, :]
                ib0 = n * 512
                psz = ps.tile([65, 512], f32, tag="z", bufs=2, name="psz")
                nplain = 4 * n
                units = []  # list of (qk_closure, av_closure)

                def mk_pair(jb):
                    ex_box = []

                    def qk():
                        pss = ps.tile([128, 1024], f32, tag="s2", bufs=3, name="pss")
                        for u in range(2):
                            nc.tensor.matmul(
                                pss[:, u * 512 : (u + 1) * 512],
                                kTh[:, (jb + u) * 128 : (jb + u + 1) * 128],
                                qTh[:, ib0 : ib0 + 512],
                                start=True,
                                stop=True,
                            )
                        ex = wp.tile([128, 1024], bf16, tag="ex", name="ex", bufs=6)
                        nc.scalar.activation(ex, pss, Exp, scale=0.125)
                        ex_box.append(ex)

                    def av():
                        ex = ex_box[0]
                        for u in range(2):
                            nc.tensor.matmul(
                                psz,
                                vv[jb + u][:, h, :],
                                ex[:, u * 512 : (u + 1) * 512],
                                start=(jb + u == 0),
                                stop=False,
                            )
                    return qk, av

                def mk_band_a():
                    jb = nplain
                    ex_box = []

                    def qk():
                        A = ps.tile([128, 1024], f32, tag="s2", bufs=3, name="A")
                        nc.tensor.matmul(
                            A[:, 0:512],
                            kTh[:, jb * 128 : (jb + 1) * 128],
                            qTh[:, ib0 : ib0 + 512],
                            start=True,
                            stop=True,
                        )
                        nc.tensor.matmul(
                            A[:, 512:896],
                            kTh[:, (jb + 1) * 128 : (jb + 2) * 128],
                            qTh[:, ib0 + 128 : ib0 + 512],
                            start=True,
                            stop=True,
                        )
                        Av = A.rearrange("p (t c) -> p t c", c=512)[:, :, 0:128]
                        nc.vector.tensor_add(Av, Av, cm2)
                        exA = wp.tile([128, 896], bf16, tag="exb", name="exA", bufs=4)
                        nc.scalar.activation(exA, A[:, 0:896], Exp, scale=0.125)
                        ex_box.append(exA)

                    def av():
                        exA = ex_box[0]
                        nc.tensor.matmul(
                            psz,
                            vv[jb][:, h, :],
                            exA[:, 0:512],
                            start=(jb == 0),
                            stop=False,
                        )
                        nc.tensor.matmul(
                            psz[:, 128:512],
                            vv[jb + 1][:, h, :],
                            exA[:, 512:896],
                            start=False,
                            stop=False,
                        )
                    return qk, av

                def mk_band_b():
                    jb = nplain + 2
                    ex_box = []

                    def qk():
                        Bt = ps.tile([128, 1024], f32, tag="s2", bufs=3, name="Bt")
                        nc.tensor.matmul(
                            Bt[:, 0:256],
                            kTh[:, jb * 128 : (jb + 1) * 128],
                            qTh[:, ib0 + 256 : ib0 + 512],
                            start=True,
                            stop=True,
                        )
                        nc.tensor.matmul(
                            Bt[:, 256:384],
                            kTh[:, (jb + 1) * 128 : (jb + 2) * 128],
                            qTh[:, ib0 + 384 : ib0 + 512],
                            start=True,
                            stop=True,
                        )
                        Bv = Bt[:, 0:512].rearrange("p (t c) -> p t c", c=256)[
                            :, :, 0:128
                        ]
                        nc.vector.tensor_add(Bv, Bv, cm2)
                        exB = wp.tile([128, 896], bf16, tag="exb", name="exB", bufs=4)
                        nc.scalar.activation(exB[:, 0:384], Bt[:, 0:384], Exp, scale=0.125)
                        ex_box.append(exB)

                    def av():
                        exB = ex_box[0]
                        nc.tensor.matmul(
                            psz[:, 256:512],
                            vv[jb][:, h, :],
                            exB[:, 0:256],
                            start=False,
                            stop=False,
                        )
                        nc.tensor.matmul(
                            psz[:, 384:512],
                            vv[jb + 1][:, h, :],
                            exB[:, 256:384],
                            start=False,
                            stop=True,
                        )
                    return qk, av

                for jb in range(0, nplain, 2):
                    units.append(mk_pair(jb))
                units.append(mk_band_a())
                units.append(mk_band_b())

                flush_at = min(3, len(units) - 1)
                for ui, (qk, _) in enumerate(units):
                    qk()
                    if ui == 0:
                        run_carry()
                    else:
                        units[ui - 1][1]()
                    inject()
                    if ui == flush_at:
                        flush_norm()

                def tail():
                    units[-1][1]()
                    dn = wp.tile([1, 512], bf16, tag="dn", bufs=3, name="dn")
                    nc.vector.tensor_copy(dn, psz[64:65, :])
                    pending.append(mk_norm(m, po, n, psz, dn))

                carry.append(tail)

            # ---- output projection for one 128-row s-tile ------------------
            def oproj(s):
                psy2 = ps.tile([128, 1024], f32, tag="s2", bufs=3, name="psy2")
                for n2 in range(2):
                    for kk in range(2):
                        nc.tensor.matmul(
                            psy2[:, n2 * 512 : (n2 + 1) * 512],
                            zT[kk][:, s * 128 : (s + 1) * 128],
                            wot[kk][:, n2 * 512 : (n2 + 1) * 512],
                            start=(kk == 0),
                            stop=(kk == 1),
                        )
                ysb = wp.tile([128, DIM], bf16, tag="ysb", bufs=3, name="ysb")
                nc.scalar.copy(ysb[:, 0:512], psy2[:, 0:512])
                nc.vector.tensor_copy(ysb[:, 512:1024], psy2[:, 512:1024])
                (sq if s % 2 == 0 else gq).dma_start(out=y_r[s], in_=ysb)

            # ---- main pipelined loop ---------------------------------------
            def act_dma(dst, src):
                def go():
                    nc.scalar.dma_start(out=dst, in_=src)
                return go

            for u in proj_units(0):
                u()

            opq = []
            for n in range(NB):
                # aux DMA issues (Act queue, delayed by its in-order position)
                # + projection units of block n+1, injected into this block's
                # attention streams
                nxt = []
                if n == 0:
                    nxt.append(act_dma(wot[0], woT_r[0]))
                    nxt.append(act_dma(wot[1], woT_r[1]))
                if n + 2 < NB:
                    nxt.append(
                        act_dma(xsb[n + 2], xc.ap()[:, n + 2])
                    )
                if n + 1 < NB:
                    nxt.extend(proj_units(n + 1))
                n_units = 4 * (2 * n + 2)
                frac = [0.0]

                def inject(nxt=nxt, n_units=n_units, frac=frac, total=len(nxt)):
                    frac[0] += total / n_units
                    while nxt and frac[0] >= 1.0:
                        frac[0] -= 1.0
                        nxt.pop(0)()

                for h in range(HG):
                    stream(h, n, inject)
                    if n > 0:
                        opq.append(4 * (n - 1) + h)
                        if len(opq) >= 2:
                            oproj(opq.pop(0))
                while nxt:
                    nxt.pop(0)()

            while opq:
                oproj(opq.pop(0))
            run_carry()
            flush_norm()
            for k in range(4):
                oproj(12 + k)

    nc.compile()
    return nc


def _get_nc():
    if "nc" not in _CACHE:
        _CACHE["nc"] = _build()
    return _CACHE["nc"]


def _in_maps(x, mask, wq, wk, wv, wo):
    bf = ml_dtypes.bfloat16
    cm8 = np.ascontiguousarray(8.0 * np.asarray(mask)[0, 0, :128, :128].T, np.float32)
    maps = []
    for c in range(NCORES):
        b, g = divmod(c, HG)
        sl = slice(OG * g, OG * (g + 1))
        # xc[p, n, e, c] = x[b][512n+c, 128e+p]
        xT = np.asarray(x)[b].T.astype(bf)  # [1024, 2048]
        xcm = np.ascontiguousarray(
            xT.reshape(8, 128, NB, 512).transpose(1, 2, 0, 3)
        )
        # wkq[p, g, e, :]: g0/g1 = wk m-chunks, g2/g3 = wq m-chunks
        wkT = np.asarray(wk)[sl, :].T.astype(bf)  # [1024, 256]
        wqT = np.asarray(wq)[sl, :].T.astype(bf)
        wkqm = np.ascontiguousarray(
            np.stack(
                [wkT[:, 0:128], wkT[:, 128:256], wqT[:, 0:128], wqT[:, 128:256]],
                axis=0,
            )
            .reshape(4, 8, 128, 128)
            .transpose(2, 0, 1, 3)
        )
        wvT = np.asarray(wv)[sl, :].T.astype(bf)  # [1024, 256]
        wv2m = np.ascontiguousarray(wvT.reshape(8, 128, 256).transpose(1, 0, 2))
        maps.append(
            {
                "xc": xcm,
                "wkq": wkqm,
                "wv2": wv2m,
                "woT": np.ascontiguousarray(np.asarray(wo)[:, sl].T).astype(bf),
                "cmask8": cm8,
            }
        )
    return maps


def _combine(results):
    y = np.zeros((B, S, DIM), np.float32)
    for c in range(NCORES):
        y[c // HG] += results[c]["y"].astype(np.float32)
    return y


def kernel(x, mask, wq, wk, wv, wo, **run_kwargs):
    from concourse.bass_utils import run_bass_kernel_spmd

    nc = _get_nc()
    res = run_bass_kernel_spmd(
        nc, _in_maps(x, mask, wq, wk, wv, wo), core_ids=list(range(NCORES)),
        **run_kwargs,
    )
    out = _combine(res.results)
    if run_kwargs:
        _CACHE["last_result"] = res
    return out
